# revision 3
# baseline (speedup 1.0000x reference)
"""Block-sparse (tridiagonal-band) attention on 8 trn2 NeuronCores.

Contract: kernel(q, k, v, mask) -> (output, attn) matching reference.py:
  q,k,v: (2,16,2048,64) f32; mask: (2,1,1,2048) bool
  output: (2,16,2048,64) f32; attn: (2,16,2048,2048) f32 (dense, block-tridiag)

Sharding: batch*heads (32) split 4-per-core across 8 cores; fully data
parallel, no collectives.

Per (b,h) device algorithm, per 64-row query block g (band = k-blocks
g-1..g+1):
  scores  s[64,<=192] = qhat_g^T-slice  @ khat_band   (one matmul; qhat has a
          ones-row and khat a mask-bias row, so scale+mask ride the matmul)
  e = exp(s) with row-sum via ACT accum_out; rden = 1/sum
  attn band = e * rden  -> staged into a [64, 2, W] window tile per 128-row
          stripe, DMA'd out together with big zero-fill DMAs from a
          persistent zero tile (attn is ~91% zeros and must be written dense)
  PV: transposed scores computed directly (3 small matmuls with kt as
          stationary), exp'd, then 3 accumulating matmuls against v tiles;
          output normalized by rden (per-partition scalar).
"""

import numpy as np

B, H, T, D = 2, 16, 2048, 64
BLOCK = 64
NB = T // BLOCK            # 32 blocks
NCORES = 8
BHPC = (B * H) // NCORES   # 4 (b,h) pairs per core
Dp = D + 1                 # contraction dim with ones/bias row
NEG = -1e9
SCALE = 1.0 / np.sqrt(np.float32(D))
ZW = T - 3 * BLOCK         # 1856: max zero-run width in a 128-row stripe

_prog_cache = {}


def _split_multi_waits(nc, mybir):
    """This container's walrus build rejects >1 sync-wait per instruction
    ("Too many sync wait commands"). Hoist all but one wait onto
    single-wait NOPs placed just before the instruction on the same
    engine stream."""
    n = 0
    for f in nc.m.functions:
        for bb in f.blocks:
            new_insts = []
            for inst in bb.instructions:
                si = getattr(inst, "sync_info", None)
                if si is not None and si.on_wait and len(si.on_wait) > 1:
                    for w in si.on_wait[:-1]:
                        new_insts.append(
                            mybir.InstNoOp(
                                name=f"I-{nc.next_id()}",
                                engine=inst.engine,
                                sync_info=mybir.SyncInfo(on_wait=[w], on_update=[]),
                                bass_nofuse=True,
                            )
                        )
                        n += 1
                    si.on_wait = si.on_wait[-1:]
                new_insts.append(inst)
            bb.instructions[:] = new_insts
    return n


def _build_program(split_waits=True):
    import concourse.bass as bass
    import concourse.tile as tile
    from concourse import mybir

    f32 = mybir.dt.float32
    Exp = mybir.ActivationFunctionType.Exp

    nc = bass.Bass()
    qt = nc.declare_dram_parameter("qt", [BHPC, Dp, T], f32, isOutput=False)
    kt = nc.declare_dram_parameter("kt", [BHPC, Dp, T], f32, isOutput=False)
    vt = nc.declare_dram_parameter("vt", [BHPC, D, T], f32, isOutput=False)
    outp = nc.declare_dram_parameter("outp", [BHPC, D, T], f32, isOutput=True)
    attn = nc.declare_dram_parameter("attn", [BHPC, T, T], f32, isOutput=True)

    with tile.TileContext(nc) as tc:
        with (
            tc.tile_pool(name="const", bufs=1) as constp,
            tc.tile_pool(name="io", bufs=2) as io,
            tc.tile_pool(name="winp", bufs=4) as winp,
            tc.tile_pool(name="sm", bufs=4) as smp,
            tc.tile_pool(name="ps", bufs=2, space="PSUM") as psp,
        ):
            zero = constp.tile([128, ZW], f32)
            nc.vector.memset(zero, 0.0)
            zbias = constp.tile([D, 1], f32)
            nc.vector.memset(zbias, 0.0)

            for bh in range(BHPC):
                qt_sb = io.tile([Dp, T], f32, tag="qt")
                nc.sync.dma_start(out=qt_sb, in_=qt[bh])
                kt_sb = io.tile([Dp, T], f32, tag="kt")
                nc.sync.dma_start(out=kt_sb, in_=kt[bh])
                vt_sb = io.tile([D, T], f32, tag="vt")
                nc.sync.dma_start(out=vt_sb, in_=vt[bh])
                out_sb = io.tile([D, T], f32, tag="out")

                for t in range(NB // 2):  # 16 stripes of 128 attn rows
                    c0 = max(0, BLOCK * (2 * t - 1))
                    c1 = min(T, BLOCK * (2 * t + 3))
                    W = c1 - c0
                    win = winp.tile([D, 2, W], f32, tag="win")
                    nc.gpsimd.memset(win, 0.0)

                    for r in range(2):
                        g = 2 * t + r
                        j0, j1 = max(0, g - 1), min(NB, g + 2)
                        nj = j1 - j0
                        BW = BLOCK * nj
                        bc0 = BLOCK * j0
                        qs = qt_sb[:, BLOCK * g : BLOCK * (g + 1)]

                        s_ps = psp.tile([D, 3 * BLOCK], f32, tag="s")
                        nc.tensor.matmul(
                            s_ps[:, :BW], qs, kt_sb[:, bc0 : bc0 + BW],
                            start=True, stop=True,
                        )
                        e = smp.tile([D, 3 * BLOCK], f32, tag="e")
                        den = smp.tile([D, 1], f32, tag="den")
                        nc.scalar.activation(
                            out=e[:, :BW], in_=s_ps[:, :BW], func=Exp,
                            bias=zbias, accum_out=den,
                        )
                        rden = smp.tile([D, 1], f32, tag="rden")
                        nc.vector.reciprocal(rden, den)
                        off = bc0 - c0
                        nc.vector.tensor_scalar_mul(
                            out=win[:, r, off : off + BW], in0=e[:, :BW], scalar1=rden,
                        )

                        sT_ps = psp.tile([D, 3 * BLOCK], f32, tag="sT")
                        for c in range(nj):
                            j = j0 + c
                            nc.tensor.matmul(
                                sT_ps[:, BLOCK * c : BLOCK * (c + 1)],
                                kt_sb[:, BLOCK * j : BLOCK * (j + 1)], qs,
                                start=True, stop=True,
                            )
                        eT = smp.tile([D, 3 * BLOCK], f32, tag="eT")
                        nc.scalar.activation(
                            out=eT[:, :BW], in_=sT_ps[:, :BW], func=Exp, bias=zbias,
                        )
                        o_ps = psp.tile([D, BLOCK], f32, tag="o")
                        for c in range(nj):
                            j = j0 + c
                            nc.tensor.matmul(
                                o_ps, eT[:, BLOCK * c : BLOCK * (c + 1)],
                                vt_sb[:, BLOCK * j : BLOCK * (j + 1)],
                                start=(c == 0), stop=(c == nj - 1),
                            )
                        nc.vector.tensor_scalar_mul(
                            out=out_sb[:, BLOCK * g : BLOCK * (g + 1)],
                            in0=o_ps, scalar1=rden,
                        )

                    rows = attn[bh, 128 * t : 128 * (t + 1), :]
                    wdst = rows[:, c0:c1].rearrange("(r p) w -> p r w", p=D)
                    nc.sync.dma_start(out=wdst, in_=win)
                    if c0 > 0:
                        nc.sync.dma_start(out=rows[:, 0:c0], in_=zero[:, :c0])
                    if c1 < T:
                        nc.sync.dma_start(out=rows[:, c1:T], in_=zero[:, : T - c1])

                nc.sync.dma_start(out=outp[bh], in_=out_sb)

    if split_waits:
        _split_multi_waits(nc, mybir)
    return nc


def _get_program():
    if "nc" not in _prog_cache:
        _prog_cache["nc"] = _build_program()
    return _prog_cache["nc"]


def _prep_inputs(q, k, v, mask):
    """Host-side shard + layout prep (numpy only)."""
    qf = np.ascontiguousarray(np.asarray(q, np.float32)).reshape(B * H, T, D)
    kf = np.ascontiguousarray(np.asarray(k, np.float32)).reshape(B * H, T, D)
    vf = np.ascontiguousarray(np.asarray(v, np.float32)).reshape(B * H, T, D)
    m = np.asarray(mask, bool).reshape(B, T)
    bias = np.where(m, np.float32(0.0), np.float32(NEG)).astype(np.float32)  # (B, T)

    in_maps = []
    for c in range(NCORES):
        sl = slice(c * BHPC, (c + 1) * BHPC)
        bidx = np.arange(c * BHPC, (c + 1) * BHPC) // H
        qt = np.empty((BHPC, Dp, T), np.float32)
        qt[:, :D, :] = qf[sl].transpose(0, 2, 1)
        qt[:, D, :] = 1.0
        kt = np.empty((BHPC, Dp, T), np.float32)
        kt[:, :D, :] = (kf[sl] * SCALE).transpose(0, 2, 1)
        kt[:, D, :] = bias[bidx]
        vt = np.ascontiguousarray(
            vf[sl].reshape(BHPC, NB, BLOCK, D).transpose(0, 2, 1, 3)
        ).reshape(BHPC, BLOCK, NB * D)
        in_maps.append(
            {"qt": np.ascontiguousarray(qt), "kt": np.ascontiguousarray(kt), "vt": vt}
        )
    return in_maps


LAST_RESULTS = None


def kernel(q, k, v, mask):
    global LAST_RESULTS
    import os
    from concourse.bass_utils import run_bass_kernel_spmd

    nc = _get_program()
    in_maps = _prep_inputs(q, k, v, mask)
    trace = bool(os.environ.get("BASS_TRACE"))
    res = run_bass_kernel_spmd(nc, in_maps, list(range(NCORES)), trace=trace)
    LAST_RESULTS = res

    outs = np.empty((B * H, T, D), np.float32)
    attns = np.empty((B * H, T, T), np.float32)
    for c in range(NCORES):
        sl = slice(c * BHPC, (c + 1) * BHPC)
        o = res.results[c]["outp"]  # (BHPC, 64, 2048)
        outs[sl] = (
            o.reshape(BHPC, BLOCK, NB, D).transpose(0, 2, 1, 3).reshape(BHPC, T, D)
        )
        attns[sl] = res.results[c]["attn"]
    return outs.reshape(B, H, T, D), attns.reshape(B, H, T, T)
